# revision 34
# baseline (speedup 1.0000x reference)
"""Trainium2 Bass kernel for nn_DCNConvModule (modulated deformable conv
+ GroupNorm(1) + ReLU).

Sharding: 8 cores; core (2b + h) computes sample b, output rows [32h, 32h+32).
GroupNorm statistics are per-sample -> tiny AllReduce of (sum, sumsq) within
core pairs [[0,1],[2,3],[4,5],[6,7]].

Per-core algorithm (pixel-major "px" = 2048 output pixels on 16 tiles of 128):
  1. offset conv as pixel-major PE matmuls -> offT [px, 27] directly
     (lhsT = fp16 x-slab window, rhs = fp16 conv_offset weights).
  2. coef/index math on DVE in fp32, run per half so half-0 transposes are
     not gated on half-1's offset conv; a zero-padded "quad" gather table in
     DRAM (one row = the 4 bilinear corner cells, 1024 els) makes corner
     validity masking implicit. The table is fp8-e3m4: halves gather DMA
     (the dominant traffic, 36.9 -> 18.4 MB/core); measured end-to-end rel
     err 1.26e-2 vs the 2e-2 gate (fp16 coefs; mixed f8xf16 matmuls).
  3. per tap: one dma_gather fetches all 4 corners of 1024 pixels; the
     bilinear+mask combine is fused into PE transposes as diagonal-matrix
     matmuls (diag(coef) built with one 4x-mode tensor_scalar each, coefs
     from the packed cAll[p, k, t, cn] tile), with the 4 corners accumulated
     in PSUM; all PSUM->SBUF valT drains on Act (DVE holds the diag builds);
     4 fp16 matmuls per (tile, tap) accumulate y in PSUM.
  4. GN: per-partition sums via ACT accumulators, ones-matmul partition
     reduce, pair AllReduce, normalize+ReLU as one ACT op per chunk.
"""
import contextlib
import numpy as np

K = 3
KK = 9
C = 256
CO = 256
H = 64
W = 64
B = 4
GW = 66                  # padded grid width
TQ = 4416                # quad-table rows (>= 65*66+65+1; TQ+67 <= 68*66)
PXT = 16                 # 128-pixel tiles per core
NPX = PXT * 128          # 2048 pixels per core
GN_EPS = 1e-5
NCORES = 8

_cache = {}


# ----------------------------------------------------------------- host prep
def prep_per_core(x, w_off, b_off, w, b, gamma, beta):
    """Build the 8 per-core input maps (all numpy, layout-only work)."""
    ki = np.arange(KK) // K
    kj = np.arange(KK) % K

    # conv_offset lhsT  [128, 2, 9, 27]: [c', cc, tap, o]
    woff_r = np.ascontiguousarray(
        w_off.reshape(27, 2, 128, K, K).transpose(2, 1, 3, 4, 0)
        .reshape(128, 2, 9, 27)).astype(np.float16)
    # main DCN lhsT  [128, 9, 2, 2, 128]: [c', k, cc, oh, o']
    w2 = np.ascontiguousarray(
        w.reshape(2, 128, 2, 128, KK).transpose(3, 4, 2, 0, 1)
    ).astype(np.float16)
    bvec = np.ascontiguousarray(b.reshape(2, 128).T).astype(np.float32)
    gam2 = np.ascontiguousarray(gamma.reshape(2, 128).T).astype(np.float32)
    bet2 = np.ascontiguousarray(beta.reshape(2, 128).T).astype(np.float32)

    p = np.arange(128)
    # dy/dx conv biases folded into the base sampling coordinates
    basex = (p[:, None] % 64 + kj[None, :]
             + b_off[2 * np.arange(KK) + 1][None, :]).astype(np.float32)
    # mask conv bias, replicated across partitions
    bmrep = np.broadcast_to(b_off[18:27][None, :], (128, 9)).astype(np.float32)
    bmrep = np.ascontiguousarray(bmrep)

    x_pad = np.zeros((B, C, H + 2, W + 2), np.float32)
    x_pad[:, :, 1:H+1, 1:W+1] = x

    in_maps = []
    for core in range(NCORES):
        bi, hh = core // 2, core % 2
        h0 = hh * 32
        # quad table: row i = cells [i, i+1, i+66, i+67] of the flat padded
        # grid (row-major 66 wide, rows 66-67 zero) -> all 4 bilinear
        # corners of (y0, x0) with implicit zero at every wrap.
        import ml_dtypes
        grid = np.zeros((68, GW, C), np.float32)
        grid[:66, :, :] = x_pad[bi].transpose(1, 2, 0)
        flat = grid.reshape(68 * GW, C)
        quad = np.concatenate(
            [flat[0:TQ], flat[1:TQ+1], flat[66:TQ+66], flat[67:TQ+67]],
            axis=1).astype(ml_dtypes.float8_e3m4)
        slab = x_pad[bi][:, h0:h0+34, :]                    # [256, 34, 66]
        # three column-shifted 64-wide copies: any (row-pair, tap) window of
        # 128 pixels is then contiguous (stationary matmul operands must
        # have a single free dim)
        sl3 = np.stack([slab[:, :, bb:bb+64] for bb in range(3)], axis=1)
        xcf = np.ascontiguousarray(
            sl3.reshape(2, 128, 3, 34, 64).transpose(1, 0, 2, 3, 4)
        ).astype(np.float16)                                # [128,2,3,34,64]
        xca = np.ascontiguousarray(xcf[:, :, :, 0:18].reshape(128, 2, 3, -1))
        xcb = np.ascontiguousarray(xcf[:, :, :, 16:34].reshape(128, 2, 3, -1))
        t = np.arange(PXT)
        basey = np.ascontiguousarray(
            (h0 + (t[None, :, None] * 128 + p[:, None, None]) // 64
             + ki[None, None, :])
            + b_off[2 * np.arange(KK)][None, None, :]).astype(np.float32)
        # rep64[q, r, m] = 1 iff q == r*16 + m%16: K=64 wrap-replicate
        # selectors (matmul base partitions are limited to {0, 32, 64})
        rep = np.zeros((128, 4, 128), np.float32)
        for wq in range(2):
            for r in range(4):
                rep[wq * 64 + r * 16 + np.arange(128) % 16, r,
                    np.arange(128)] = 1.0
        in_maps.append(dict(
            xt=np.ascontiguousarray(quad), xca=xca, xcb=xcb,
            woff=woff_r, w2=w2,
            bvec=bvec, gam2=gam2, bet2=bet2, bmrep=bmrep,
            basey=basey, basex=basex, rep16=rep,
        ))
    return in_maps


# --------------------------------------------------------------- bass kernel
def build_module(use_collective=True):
    import concourse.bacc as bacc
    import concourse.bass as bass
    import concourse.tile as tile
    from concourse import mybir
    from concourse.masks import make_identity

    f32 = mybir.dt.float32
    f16 = mybir.dt.float16
    f8 = mybir.dt.float8e3
    i16 = mybir.dt.int16
    Alu = mybir.AluOpType
    Act = mybir.ActivationFunctionType

    nc = bacc.Bacc("TRN2", target_bir_lowering=False, debug=False,
                   num_devices=NCORES)

    xt = nc.dram_tensor("xt", [TQ, 1024], f8, kind="ExternalInput")
    xca = nc.dram_tensor("xca", [128, 2, 3, 18 * 64], f16, kind="ExternalInput")
    xcb = nc.dram_tensor("xcb", [128, 2, 3, 18 * 64], f16, kind="ExternalInput")
    woff = nc.dram_tensor("woff", [128, 2, 9, 27], f16, kind="ExternalInput")
    w2 = nc.dram_tensor("w2", [128, 9, 2, 2, 128], f16, kind="ExternalInput")
    bvec = nc.dram_tensor("bvec", [128, 2], f32, kind="ExternalInput")
    gam2 = nc.dram_tensor("gam2", [128, 2], f32, kind="ExternalInput")
    bet2 = nc.dram_tensor("bet2", [128, 2], f32, kind="ExternalInput")
    bmrep = nc.dram_tensor("bmrep", [128, 9], f32, kind="ExternalInput")
    basey = nc.dram_tensor("basey", [128, PXT, 9], f32, kind="ExternalInput")
    basex = nc.dram_tensor("basex", [128, 9], f32, kind="ExternalInput")
    rep16 = nc.dram_tensor("rep16", [128, 4, 128], f32, kind="ExternalInput")
    yout = nc.dram_tensor("yout", [CO, NPX], f16, kind="ExternalOutput")

    cc_in = nc.dram_tensor("cc_in", [1, 8], f32)
    cc_out = nc.dram_tensor("cc_out", [1, 8], f32)

    xt_win = bass.AP(tensor=xt, offset=0, ap=[[1024, TQ], [1, 1024]])

    def swap_free(ap2):
        """Swap the two free dims of a [P, A, B] AP (iteration order only)."""
        return bass.AP(tensor=ap2.tensor, offset=ap2.offset,
                       ap=[ap2.ap[0], ap2.ap[2], ap2.ap[1]])

    def bcast_free(ap1, n):
        """View a [P, F] AP as [P, n, F] with stride-0 broadcast."""
        return bass.AP(tensor=ap1.tensor, offset=ap1.offset,
                       ap=[ap1.ap[0], [0, n], ap1.ap[1]])

    def mk_ap(ap0, off_els, dims):
        """Custom free dims [stride, n] on ap0's tensor (partition dim kept)."""
        return bass.AP(tensor=ap0.tensor, offset=ap0.offset + off_els,
                       ap=[ap0.ap[0]] + [list(d) for d in dims])

    with tile.TileContext(nc) as tc, contextlib.ExitStack() as ctx:
        consts = ctx.enter_context(tc.tile_pool(name="consts", bufs=1))
        sb = ctx.enter_context(tc.tile_pool(name="sb", bufs=1))
        gat = ctx.enter_context(tc.tile_pool(name="gat", bufs=4))
        diags = ctx.enter_context(tc.tile_pool(name="diags", bufs=3))
        vals = ctx.enter_context(tc.tile_pool(name="vals", bufs=4))
        ps_vt = ctx.enter_context(
            tc.tile_pool(name="ps_vt", bufs=4, space="PSUM"))
        ps_y = ctx.enter_context(
            tc.tile_pool(name="ps_y", bufs=1, space="PSUM"))

        ident16 = consts.tile([128, 128], f16)
        make_identity(nc, ident16[:])
        ones_row = consts.tile([1, 128], f32)
        nc.vector.memset(ones_row[:], 1.0)
        ones_col = consts.tile([128, 1], f32)
        nc.vector.memset(ones_col[:], 1.0)
        eps_t = consts.tile([1, 1], f32)
        nc.vector.memset(eps_t[:], GN_EPS)

        # load order: xc+woff gate phase 1, basey/basex/rep16 gate the index
        # path to the first gather; w2 is not needed until the main matmuls
        woff_sb = consts.tile([128, 2, 9, 27], f16)
        nc.sync.dma_start(out=woff_sb[:], in_=woff[:])
        # chunked slab load: first chunk covers tiles 0-3 so phase 1 starts
        # before the rest of the slab lands
        xca_sb = consts.tile([128, 2, 3, 18 * 64], f16)
        nc.sync.dma_start(out=xca_sb[:, :, :, 0:640], in_=xca[:, :, :, 0:640])
        basey_sb = consts.tile([128, PXT, 9], f32)
        nc.sync.dma_start(out=basey_sb[:], in_=basey[:])
        basex_sb = consts.tile([128, 9], f32)
        nc.sync.dma_start(out=basex_sb[:], in_=basex[:])
        rep16_sb = consts.tile([128, 4, 128], f32)
        nc.sync.dma_start(out=rep16_sb[:], in_=rep16[:])
        bmrep_sb = consts.tile([128, 9], f32)
        nc.sync.dma_start(out=bmrep_sb[:], in_=bmrep[:])
        nc.sync.dma_start(out=xca_sb[:, :, :, 640:1152],
                          in_=xca[:, :, :, 640:1152])
        xcb_sb = consts.tile([128, 2, 3, 18 * 64], f16)
        nc.sync.dma_start(out=xcb_sb[:], in_=xcb[:])
        bvec_sb = consts.tile([128, 2], f32)
        nc.sync.dma_start(out=bvec_sb[:], in_=bvec[:])
        gam_sb = consts.tile([128, 2], f32)
        nc.sync.dma_start(out=gam_sb[:], in_=gam2[:])
        bet_sb = consts.tile([128, 2], f32)
        nc.sync.dma_start(out=bet_sb[:], in_=bet2[:])
        w2_sb = consts.tile([128, 9, 2, 2, 128], f16)
        nc.sync.dma_start(out=w2_sb[:], in_=w2[:])

        # ---------------- phase 2b: bilinear corner coefficients -----------
        # run per half (inside the phase-1 loop) so half-0 transposes are not
        # gated on half-1's offset conv
        mss = sb.tile([128, PXT, 9], f32)
        msk = sb.tile([128, PXT, 9], f32)
        ly = sb.tile([128, PXT, 9], f32)
        lx = sb.tile([128, PXT, 9], f32)
        ly1 = sb.tile([128, PXT, 9], f32)
        lx1 = sb.tile([128, PXT, 9], f32)
        ay0 = sb.tile([128, PXT, 9], f32)
        ay1 = sb.tile([128, PXT, 9], f32)
        c01t = sb.tile([128, PXT, 9], f32)
        c10t = sb.tile([128, PXT, 9], f32)
        c11t = sb.tile([128, PXT, 9], f32)
        mxv = sb.tile([128, PXT, 9], f32)
        myv = sb.tile([128, PXT, 9], f32)
        # packed coefs  cAll[128, k 9, t 16, cn 4] f32: scalar column (k,t,cn)
        # feeds each diag build
        cAll = sb.tile([128, 9, PXT, 4], f32)

        def coef_half(h):
            hs = slice(h * 8, h * 8 + 8)

            def call_slice(cn):
                return mk_ap(cAll[:], cn + h * 32, [[4, 8], [64, 9]])

            nc.vector.tensor_tensor(out=mss[:, hs], in0=offT[:, hs, 18:27],
                                    in1=bcast_free(bmrep_sb[:], 8), op=Alu.add)
            nc.scalar.activation(out=msk[:, hs], in_=mss[:, hs],
                                 func=Act.Sigmoid)
            nc.vector.tensor_tensor(out=ly[:, hs], in0=pyg[:, hs],
                                    in1=y0f[:, hs], op=Alu.subtract)
            nc.vector.tensor_tensor(out=lx[:, hs], in0=pxg[:, hs],
                                    in1=x0f[:, hs], op=Alu.subtract)
            nc.vector.tensor_scalar(out=ly1[:, hs], in0=ly[:, hs], scalar1=-1.0,
                                    scalar2=1.0, op0=Alu.mult, op1=Alu.add)
            nc.vector.tensor_scalar(out=lx1[:, hs], in0=lx[:, hs], scalar1=-1.0,
                                    scalar2=1.0, op0=Alu.mult, op1=Alu.add)
            nc.vector.tensor_tensor(out=ay0[:, hs], in0=ly1[:, hs],
                                    in1=msk[:, hs], op=Alu.mult)
            nc.vector.tensor_tensor(out=ay1[:, hs], in0=ly[:, hs],
                                    in1=msk[:, hs], op=Alu.mult)
            nc.vector.tensor_tensor(out=call_slice(0), in0=ay0[:, hs],
                                    in1=lx1[:, hs], op=Alu.mult)
            nc.vector.tensor_tensor(out=c01t[:, hs], in0=ay0[:, hs],
                                    in1=lx[:, hs], op=Alu.mult)
            nc.vector.tensor_tensor(out=c10t[:, hs], in0=ay1[:, hs],
                                    in1=lx1[:, hs], op=Alu.mult)
            nc.vector.tensor_tensor(out=c11t[:, hs], in0=ay1[:, hs],
                                    in1=lx[:, hs], op=Alu.mult)
            # clamping maps x0<=-2 (y0<=-2) pairs onto (border, image 0): the
            # second pair element reads wrong data -> kill +1-corner coefs
            nc.vector.tensor_scalar(out=mxv[:, hs], in0=x0f[:, hs], scalar1=0.0,
                                    scalar2=None, op0=Alu.is_ge)
            nc.vector.tensor_scalar(out=myv[:, hs], in0=y0f[:, hs], scalar1=0.0,
                                    scalar2=None, op0=Alu.is_ge)
            nc.vector.tensor_tensor(out=call_slice(1), in0=c01t[:, hs],
                                    in1=mxv[:, hs], op=Alu.mult)
            nc.vector.tensor_tensor(out=call_slice(2), in0=c10t[:, hs],
                                    in1=myv[:, hs], op=Alu.mult)
            nc.vector.tensor_tensor(out=c11t[:, hs], in0=c11t[:, hs],
                                    in1=mxv[:, hs], op=Alu.mult)
            nc.vector.tensor_tensor(out=call_slice(3), in0=c11t[:, hs],
                                    in1=myv[:, hs], op=Alu.mult)

        # ---- phases 1+3, pipelined per half of the pixel tiles -----------
        # half h covers pixel tiles t in [8h, 8h+8); its offset conv needs
        # only slab h, and its gather indices only its own pixels, so the
        # half-0 gathers launch while slab 1 is still loading.
        offT = sb.tile([128, PXT, 27], f32)
        pyg = sb.tile([128, PXT, 9], f32)
        pxg = sb.tile([128, PXT, 9], f32)
        ffy = sb.tile([128, PXT, 9], f32)
        ffx = sb.tile([128, PXT, 9], f32)
        y0f = sb.tile([128, PXT, 9], f32)
        x0f = sb.tile([128, PXT, 9], f32)
        x0c = sb.tile([128, PXT, 9], f32)
        wbufs = [sb.tile([128, 9, 8, 8], i16, tag="wbuf0", name="wbuf0"),
                 sb.tile([128, 9, 8, 8], i16, tag="wbuf1", name="wbuf1")]
        idxfs = [sb.tile([128, 9, 8], f32, tag="idxf0", name="idxf0"),
                 sb.tile([128, 9, 8], f32, tag="idxf1", name="idxf1")]

        for h in range(2):
            src_sl = (xca_sb, xcb_sb)[h]
            hs = slice(h * 8, h * 8 + 8)
            for t in range(h * 8, h * 8 + 8):
                ps_off = ps_vt.tile([128, 27], f32, tag="ps_v")
                n = 0
                for cc in range(2):
                    for a in range(3):
                        for bb in range(3):
                            r0 = (2 * t + a - 16 * h) * 64
                            nc.tensor.matmul(
                                ps_off[:, :],
                                src_sl[:, cc, bb, r0: r0 + 128],
                                woff_sb[:, cc, a * 3 + bb, :],
                                start=(n == 0), stop=(n == 17))
                            n += 1
                nc.scalar.copy(out=offT[:, t, :], in_=ps_off[:, :])

            # sampling coordinates + truncation-based clamped indices (for
            # py < 0 clamp(trunc) == clamp(floor), so trunc suffices here)
            nc.vector.tensor_tensor(out=pyg[:, hs, :], in0=offT[:, hs, 0:18:2],
                                    in1=basey_sb[:, hs, :], op=Alu.add)
            nc.vector.tensor_tensor(out=pxg[:, hs, :], in0=offT[:, hs, 1:18:2],
                                    in1=bcast_free(basex_sb[:], 8), op=Alu.add)
            # floor = round-to-nearest int conversion corrected by is_gt
            iiy = sb.tile([128, 8, 9], mybir.dt.int32, tag="iiy", name="iiy")
            nc.vector.tensor_copy(out=iiy[:], in_=pyg[:, hs, :])
            nc.vector.tensor_copy(out=ffy[:, hs, :], in_=iiy[:])
            gty = sb.tile([128, 8, 9], f32, tag="gty", name="gty")
            nc.vector.tensor_tensor(out=gty[:], in0=ffy[:, hs, :],
                                    in1=pyg[:, hs, :], op=Alu.is_gt)
            nc.vector.tensor_tensor(out=y0f[:, hs, :], in0=ffy[:, hs, :],
                                    in1=gty[:], op=Alu.subtract)
            iix = sb.tile([128, 8, 9], mybir.dt.int32, tag="iix", name="iix")
            nc.vector.tensor_copy(out=iix[:], in_=pxg[:, hs, :])
            nc.vector.tensor_copy(out=ffx[:, hs, :], in_=iix[:])
            gtx = sb.tile([128, 8, 9], f32, tag="gtx", name="gtx")
            nc.vector.tensor_tensor(out=gtx[:], in0=ffx[:, hs, :],
                                    in1=pxg[:, hs, :], op=Alu.is_gt)
            nc.vector.tensor_tensor(out=x0f[:, hs, :], in0=ffx[:, hs, :],
                                    in1=gtx[:], op=Alu.subtract)
            y0ch = sb.tile([128, 8, 9], f32, tag="y0ch", name="y0ch")
            nc.vector.tensor_scalar(out=y0ch[:], in0=y0f[:, hs, :],
                                    scalar1=0.0, scalar2=65.0,
                                    op0=Alu.max, op1=Alu.min)
            nc.vector.tensor_scalar(out=x0c[:, hs, :], in0=x0f[:, hs, :],
                                    scalar1=0.0, scalar2=65.0,
                                    op0=Alu.max, op1=Alu.min)
            tmpi = sb.tile([128, 8, 9], f32, tag="tmpi", name="tmpi")
            nc.vector.tensor_scalar_mul(out=tmpi[:], in0=y0ch[:], scalar1=66.0)
            nc.vector.tensor_tensor(out=swap_free(idxfs[h][:]), in0=tmpi[:],
                                    in1=x0c[:, hs, :], op=Alu.add)
            # wrap+replicate via one matmul per 16-partition group:
            # out[m, f] = idxf[u*16 + m%16, f]
            for u in range(8):
                w_, r_ = u // 4, u % 4
                ps_rep = ps_vt.tile([128, 72], f32, tag="ps_v")
                nc.tensor.matmul(ps_rep[:, :],
                                 rep16_sb[64 * w_:64 * (w_ + 1), r_, :],
                                 idxfs[h][64 * w_:64 * (w_ + 1), :, :],
                                 start=True, stop=True)
                nc.vector.tensor_copy(
                    out=wbufs[h][:, :, :, u],
                    in_=ps_rep[:, :].rearrange("p (a t) -> p a t", t=8))
            coef_half(h)

        dum = sb.tile([1, 1], f32)
        nc.scalar.activation(out=dum[:], in_=eps_t[:], func=Act.Sqrt)

        # ---------------- phase 4: main loop -------------------------------
        y16 = sb.tile([128, 2, 2, 1024], f16)       # [o', oh, half, px]
        s1b = sb.tile([128, 2, 2], f32)
        s2b = sb.tile([128, 2, 2], f32)

        y_ps = [ps_y.tile([128, 1024], f32, tag=f"y_ps{oh}", name=f"y_ps{oh}")
                for oh in range(2)]

        # GPSIMD cannot access PSUM; Act has slack (DVE is loaded with the
        # diag builds) -> put all valT drains on Act
        cp_engines = [nc.scalar.copy]
        cp_i = 0

        for half in range(2):
            for k in range(KK):
                last = (half == 1 and k == KK - 1)
                if not last:
                    g = gat.tile([128, 8, 1024], f8, tag="g")
                    nc.gpsimd.dma_gather(
                        out_ap=g[:], in_ap=xt_win,
                        idxs_ap=wbufs[half][:, k, :, :],
                        num_idxs=1024, num_idxs_reg=1024,
                        elem_size=1024, queue_num=0)
                else:
                    # final tap: two half-gathers so the drain starts earlier
                    g = gat.tile([128, 8, 1024], f8, tag="g")
                    for gh in range(2):
                        nc.gpsimd.dma_gather(
                            out_ap=g[:, gh * 4:(gh + 1) * 4, :], in_ap=xt_win,
                            idxs_ap=wbufs[half][:, k, gh * 4:(gh + 1) * 4, :],
                            num_idxs=512, num_idxs_reg=512,
                            elem_size=1024, queue_num=0)
                valTs = []
                for t8p in range(4):                 # pairs of px tiles
                    ps_v = ps_vt.tile([128, 512], f32, tag="ps_v")
                    dg = [[diags.tile([128, 128], f16, tag=f"d{j}{cn}",
                                      name=f"d{j}{cn}")
                           for cn in range(4)] for j in range(2)]
                    for j in range(2):
                        t8 = t8p * 2 + j
                        t = half * 8 + t8
                        for cn in range(4):
                            nc.vector.tensor_scalar_mul(
                                out=dg[j][cn][:], in0=ident16[:],
                                scalar1=cAll[:, k, t, cn:cn + 1])
                    for j in range(2):
                        t8 = t8p * 2 + j
                        for hh in range(2):
                            for cn in range(4):
                                nc.tensor.matmul(
                                    ps_v[:, j * 256 + hh * 128:
                                         j * 256 + (hh + 1) * 128],
                                    g[:, t8, cn * 256 + hh * 128:
                                      cn * 256 + (hh + 1) * 128],
                                    dg[j][cn][:],
                                    start=(j == 0 and hh == 0 and cn == 0),
                                    stop=(j == 1 and hh == 1 and cn == 3))
                    valT = vals.tile([128, 512], f16, tag="valT")
                    cp_engines[cp_i % len(cp_engines)](out=valT[:], in_=ps_v[:])
                    cp_i += 1
                    valTs.append(valT)
                # main matmuls after all scaled transposes: the PSUM->SBUF
                # copies drain while later pairs' transposes keep PE busy
                for t8p in range(4):
                    for j in range(2):
                        t8 = t8p * 2 + j
                        for oh in range(2):
                            for cc in range(2):
                                # start/stop are per 2KB PSUM bank (= 4 t8
                                # slices): start clears has_written for the
                                # whole bank, so only the first matmul
                                # touching the bank sets it.
                                nc.tensor.matmul(
                                    y_ps[oh][:, t8 * 128:(t8 + 1) * 128],
                                    w2_sb[:, k, cc, oh, :],
                                    valTs[t8p][:, j * 256 + cc * 128:
                                               j * 256 + (cc + 1) * 128],
                                    start=(k == 0 and cc == 0 and t8 % 4 == 0),
                                    stop=(k == KK - 1 and cc == 1
                                          and t8 % 4 == 3))
            for oh in range(2):
                sq_scratch = sb.tile([128, 1024], f16, tag="sq")
                nc.scalar.activation(out=y16[:, oh, half, :], in_=y_ps[oh][:],
                                     func=Act.Copy,
                                     accum_out=s1b[:, oh, half:half + 1])
                nc.vector.tensor_tensor(out=sq_scratch[:],
                                        in0=y16[:, oh, half, :],
                                        in1=y16[:, oh, half, :], op=Alu.mult)
                nc.vector.tensor_reduce(out=s2b[:, oh, half:half + 1],
                                        in_=sq_scratch[:],
                                        axis=mybir.AxisListType.XY,
                                        op=Alu.add)

        # ---------------- phase 5: GroupNorm -------------------------------
        s1 = sb.tile([128, 2], f32)
        nc.vector.tensor_tensor(out=s1[:], in0=s1b[:, :, 0], in1=s1b[:, :, 1],
                                op=Alu.add)
        s2 = sb.tile([128, 2], f32)
        nc.vector.tensor_tensor(out=s2[:], in0=s2b[:, :, 0], in1=s2b[:, :, 1],
                                op=Alu.add)
        # fold conv bias b: S1' = S1 + NPX*b ; S2' = S2 + 2 b S1 + NPX b^2
        stk = sb.tile([128, 4], f32)
        q1 = sb.tile([128, 2], f32)
        nc.vector.tensor_tensor(out=q1[:], in0=bvec_sb[:], in1=s1[:],
                                op=Alu.mult)
        nc.vector.scalar_tensor_tensor(out=stk[:, 2:4], in0=q1[:], scalar=2.0,
                                       in1=s2[:], op0=Alu.mult, op1=Alu.add)
        q2 = sb.tile([128, 2], f32)
        nc.vector.tensor_tensor(out=q2[:], in0=bvec_sb[:], in1=bvec_sb[:],
                                op=Alu.mult)
        nc.vector.scalar_tensor_tensor(out=stk[:, 2:4], in0=q2[:],
                                       scalar=float(NPX), in1=stk[:, 2:4],
                                       op0=Alu.mult, op1=Alu.add)
        nc.vector.scalar_tensor_tensor(out=stk[:, 0:2], in0=bvec_sb[:],
                                       scalar=float(NPX), in1=s1[:],
                                       op0=Alu.mult, op1=Alu.add)
        ps_s = ps_vt.tile([1, 4], f32, tag="ps_v")
        nc.tensor.matmul(ps_s[:, :], ones_col[:, :], stk[:, :],
                         start=True, stop=True)
        tot4 = sb.tile([1, 4], f32)
        nc.vector.tensor_copy(out=tot4[:], in_=ps_s[:, :])
        ccs = sb.tile([1, 8], f32)
        nc.vector.memset(ccs[:], 0.0)
        nc.vector.tensor_tensor(out=ccs[:, 0:1], in0=tot4[:, 0:1],
                                in1=tot4[:, 1:2], op=Alu.add)
        nc.vector.tensor_tensor(out=ccs[:, 1:2], in0=tot4[:, 2:3],
                                in1=tot4[:, 3:4], op=Alu.add)

        tot = sb.tile([1, 8], f32)
        if use_collective:
            nc.sync.dma_start(out=cc_in[:], in_=ccs[:])
            nc.gpsimd.collective_compute(
                "AllReduce", Alu.add,
                replica_groups=[[0, 1], [2, 3], [4, 5], [6, 7]],
                ins=[cc_in[:].opt()], outs=[cc_out[:].opt()])
            nc.sync.dma_start(out=tot[:], in_=cc_out[:])
        else:
            nc.vector.tensor_scalar_mul(out=tot[:], in0=ccs[:], scalar1=2.0)

        invN = 1.0 / float(C * H * W)
        mu = sb.tile([1, 1], f32)
        nc.vector.tensor_scalar_mul(out=mu[:], in0=tot[:, 0:1], scalar1=invN)
        mu2 = sb.tile([1, 1], f32)
        nc.vector.tensor_tensor(out=mu2[:], in0=mu[:], in1=mu[:], op=Alu.mult)
        var = sb.tile([1, 1], f32)
        nc.vector.scalar_tensor_tensor(out=var[:], in0=tot[:, 1:2],
                                       scalar=invN, in1=mu2[:],
                                       op0=Alu.mult, op1=Alu.subtract)
        std = sb.tile([1, 1], f32)
        nc.scalar.activation(out=std[:], in_=var[:], func=Act.Sqrt,
                             bias=eps_t[:, 0:1])
        mr = sb.tile([1, 2], f32)
        nc.vector.tensor_copy(out=mr[:, 0:1], in_=mu[:])
        nc.vector.reciprocal(out=mr[:, 1:2], in_=std[:])
        ps_b = ps_vt.tile([128, 2], f32, tag="ps_v")
        nc.tensor.matmul(ps_b[:, :], ones_row[:, :], mr[:, :],
                         start=True, stop=True)
        mr128 = sb.tile([128, 2], f32)
        nc.vector.tensor_copy(out=mr128[:], in_=ps_b[:, :])
        svec = sb.tile([128, 2], f32)
        nc.vector.tensor_scalar_mul(out=svec[:], in0=gam_sb[:],
                                    scalar1=mr128[:, 1:2])
        tdiff = sb.tile([128, 2], f32)
        nc.vector.tensor_scalar_sub(out=tdiff[:], in0=bvec_sb[:],
                                    scalar1=mr128[:, 0:1])
        b2 = sb.tile([128, 2], f32)
        nc.vector.tensor_tensor(out=b2[:], in0=tdiff[:], in1=svec[:],
                                op=Alu.mult)
        nc.vector.tensor_tensor(out=b2[:], in0=b2[:], in1=bet_sb[:],
                                op=Alu.add)

        for oh in range(2):
            for half in range(2):
                # scale+bias then relu on DVE (f16 4x mode), freeing Act
                nc.vector.tensor_scalar(out=y16[:, oh, half, :],
                                        in0=y16[:, oh, half, :],
                                        scalar1=svec[:, oh:oh + 1],
                                        scalar2=b2[:, oh:oh + 1],
                                        op0=Alu.mult, op1=Alu.add)
                nc.vector.tensor_scalar_max(out=y16[:, oh, half, :],
                                            in0=y16[:, oh, half, :],
                                            scalar1=0.0)
                nc.sync.dma_start(
                    out=yout[oh * 128:(oh + 1) * 128,
                             half * 1024:(half + 1) * 1024],
                    in_=y16[:, oh, half, :])

    nc.compile()
    return nc


# ----------------------------------------------------------------- entry
def kernel(x, w_off, b_off, w, b, gamma, beta):
    from concourse.bass_utils import run_bass_kernel_spmd

    in_maps = prep_per_core(np.asarray(x, np.float32),
                            np.asarray(w_off, np.float32),
                            np.asarray(b_off, np.float32),
                            np.asarray(w, np.float32),
                            np.asarray(b, np.float32),
                            np.asarray(gamma, np.float32),
                            np.asarray(beta, np.float32))
    if "nc" not in _cache:
        _cache["nc"] = build_module(use_collective=True)
    res = run_bass_kernel_spmd(_cache["nc"], in_maps,
                               core_ids=list(range(NCORES)))
    out = np.zeros((B, CO, H, W), np.float32)
    for core in range(NCORES):
        bi, hh = core // 2, core % 2
        out[bi, :, hh * 32:(hh + 1) * 32, :] = (
            res.results[core]["yout"].reshape(CO, 32, 64))
    return out

